# revision 37
# baseline (speedup 1.0000x reference)
"""Trainium2 Bass kernel for a dense Mamba (selective-scan) block, SPMD over 8 NeuronCores.

v3: tensor-parallel over d_inner (2048 -> 256 ch/core), software-pipelined at
t-block (1024) granularity: round r runs in_proj/conv/x_proj + per-block
AllReduce for block r while the DVE-bound scan runs block r-2. Per core:
in_proj -> z-SiLU (native Silu) -> causal conv (4 diagonal matmuls) -> SiLU ->
x_proj partial -> AllReduce (8x 384KB) -> dt softplus (Exp/Ln) -> scan per
(block, n-pair, h): dA=Exp(dts*A_n) f32, dBx pair-batched bf16 mul, hardware
tensor_tensor_scan (fp32 state), y=h*C pair-batched, n-reduction + D-skip via
PSUM-accumulated matmuls (identity + diag(D)) -> gate -> AllToAll -> out_proj
(2 dm-half passes, 8 PSUM accumulators) -> per-core t-slice, host concat.

Shapes hardcoded for: B=2, L=4096, d_model=1024, d_inner=2048, d_state=16,
d_conv=4, dt_rank=64, f32 I/O.
"""
import numpy as np
import ml_dtypes
from contextlib import ExitStack

import concourse.bass as bass
import concourse.bacc as bacc
import concourse.tile as tile
from concourse import mybir
from concourse import bass_utils

BF = ml_dtypes.bfloat16
F32 = mybir.dt.float32
BF16 = mybir.dt.bfloat16

NCORES = 8
B, L, DM = 2, 4096, 1024
DI, DS, DC, DTR = 2048, 16, 4, 64
DL = DI // NCORES          # 256 local channels
NDH = DL // 128            # 2 d-half tiles
T = B * L                  # 8192 flattened (b, l)
TSL = T // NCORES          # 1024 t-slice per core for the output
TCA = 512                  # phase A matmul chunk
TCC = 1024                 # pipeline block (== TSL)
NTCB = T // TCC            # 8
NPP = 2                    # n-pairs per (block, h) whose muls go to gpsimd
CHEAD = 2                  # n-pairs emitted before phaseB each round

AF = mybir.ActivationFunctionType

_cached = {}


def _act_table_id(nc, name):
    from concourse.hw_specs import get_activation_tables
    return list(get_activation_tables(nc.m.arch).keys()).index(name)


def _build():
    nc = bacc.Bacc("TRN2", target_bir_lowering=False, num_devices=NCORES)

    # ---- I/O -------------------------------------------------------------
    d_hT = nc.dram_tensor("hT", (DM, T), BF16, kind="ExternalInput")
    d_wxzT = nc.dram_tensor("wxzT", (DM, 2 * DL), BF16, kind="ExternalInput")
    d_cdiag = nc.dram_tensor("cdiag", (DC, NDH, 128, 128), BF16, kind="ExternalInput")
    d_convb = nc.dram_tensor("convb", (NDH, 128, 1), F32, kind="ExternalInput")
    d_xprojT = nc.dram_tensor("xprojT", (NDH, 128, DTR + 2 * DS), BF16, kind="ExternalInput")
    d_dtwT = nc.dram_tensor("dtwT", (DTR, DL), BF16, kind="ExternalInput")
    d_dtb = nc.dram_tensor("dtb", (NDH, 128, 1), F32, kind="ExternalInput")
    d_aneg = nc.dram_tensor("aneg", (NDH, 128, DS), F32, kind="ExternalInput")
    d_ddiag = nc.dram_tensor("ddiag", (NDH, 128, 128), BF16, kind="ExternalInput")
    d_woutT = nc.dram_tensor("woutT", (2 * NCORES, 128, DM), BF16, kind="ExternalInput")
    d_ident = nc.dram_tensor("ident", (128, 128), BF16, kind="ExternalInput")
    d_out = nc.dram_tensor("out_slice", (TSL, DM), F32, kind="ExternalOutput")

    # ---- internal DRAM ---------------------------------------------------
    d_xdp = nc.dram_tensor("xdp", (NTCB, DTR + 2 * DS, TCC), F32, kind="Internal")
    d_xd = nc.dram_tensor("xd", (NTCB, DTR + 2 * DS, TCC), F32, kind="Internal",
                          addr_space="Shared")
    d_zsp = nc.dram_tensor("zsp", (NDH, 128, T), BF16, kind="Internal")
    d_bc = nc.dram_tensor("bcrows", (2 * DS, T), BF16, kind="Internal")
    d_a2ai = [nc.dram_tensor(f"a2ai{h}", (NCORES, 128, TSL), BF16, kind="Internal")
              for h in range(NDH)]
    d_a2ao = [nc.dram_tensor(f"a2ao{h}", (NCORES, 128, TSL), BF16, kind="Internal")
              for h in range(NDH)]

    groups = [list(range(NCORES))]

    with tile.TileContext(nc) as tc, ExitStack() as ctx:
        consts = ctx.enter_context(tc.tile_pool(name="consts", bufs=1))
        resid = ctx.enter_context(tc.tile_pool(name="resid", bufs=1))

        # ---- load constants ----------------------------------------------
        wxz = consts.tile([128, 8, 2 * DL], BF16, tag="wxz")
        nc.sync.dma_start(out=wxz, in_=d_wxzT[:, :].rearrange("(k p) m -> p k m", p=128))
        cdg = consts.tile([128, DC, NDH, 128], BF16, tag="cdg")
        nc.sync.dma_start(
            out=cdg, in_=bass.AP(tensor=d_cdiag[:, :, :, :].tensor, offset=0,
                                 ap=[[128, 128], [NDH * 128 * 128, DC], [128 * 128, NDH], [1, 128]]))
        convb = consts.tile([128, NDH, 1], F32, tag="convb")
        nc.sync.dma_start(out=convb, in_=d_convb[:, :, :].rearrange("h p one -> p h one"))
        xprj = consts.tile([128, NDH, DTR + 2 * DS], BF16, tag="xprj")
        nc.sync.dma_start(out=xprj, in_=d_xprojT[:, :, :].rearrange("h p m -> p h m"))
        dtw = consts.tile([DTR, DL], BF16, tag="dtw")
        nc.sync.dma_start(out=dtw, in_=d_dtwT[:, :])
        dtb = consts.tile([128, NDH, 1], F32, tag="dtb")
        nc.sync.dma_start(out=dtb, in_=d_dtb[:, :, :].rearrange("h p one -> p h one"))
        aneg = consts.tile([128, NDH, DS], F32, tag="aneg")
        nc.sync.dma_start(out=aneg, in_=d_aneg[:, :, :].rearrange("h p n -> p h n"))
        ddg = consts.tile([128, NDH, 128], BF16, tag="ddg")
        nc.sync.dma_start(out=ddg, in_=d_ddiag[:, :, :].rearrange("h p q -> p h q"))
        ident = consts.tile([128, 128], BF16, tag="ident")
        nc.sync.dma_start(out=ident, in_=d_ident[:, :])
        carry = consts.tile([128, NDH, DS], F32, tag="carry")

        # ---- SBUF-resident activations -----------------------------------
        xs = resid.tile([128, NDH, T], BF16, tag="xs")        # silu(conv(x))
        dts = resid.tile([128, NDH, T], BF16, tag="dts")      # softplus dt

        with tc.tile_pool(name="xpadp", bufs=2) as xpadp, \
             tc.tile_pool(name="work", bufs=2) as work, \
             tc.tile_pool(name="workC", bufs=2) as workC, \
             tc.tile_pool(name="bcp", bufs=3) as bcp, \
             tc.tile_pool(name="psA", bufs=3, space="PSUM") as psA, \
             tc.tile_pool(name="psY", bufs=1, space="PSUM") as psY:

            pys = [[psY.tile([128, 512], F32, tag=f"py_{h}_{qq}", name=f"py_{h}_{qq}")
                    for qq in range(2)] for h in range(NDH)]
            xpads = {}

            def phaseA(tcb):
                """in_proj + conv + x_proj partial + AllReduce for block tcb."""
                t0 = tcb * TCC
                xpad = xpadp.tile([128, NDH, 3 + TCC], BF16, tag="xpad",
                                  name=f"xpad{tcb}")
                xpads[tcb] = xpad
                # in_proj: 2 chunks of 512
                for cc in range(2):
                    tc0 = t0 + cc * TCA
                    ht = work.tile([128, 8, TCA], BF16, tag="ht")
                    nc.sync.dma_start(
                        out=ht,
                        in_=bass.AP(tensor=d_hT[:, :].tensor, offset=tc0,
                                    ap=[[T, 128], [128 * T, 8], [1, TCA]]))
                    for m in range(4):  # 0,1: x halves; 2,3: z halves
                        pxz = psA.tile([128, TCA], F32, tag="ps")
                        for k in range(8):
                            nc.tensor.matmul(pxz, lhsT=wxz[:, k, m * 128:(m + 1) * 128],
                                             rhs=ht[:, k, :], start=(k == 0), stop=(k == 7))
                        if m < 2:
                            nc.scalar.copy(
                                xpad[:, m, 3 + cc * TCA: 3 + (cc + 1) * TCA], pxz)
                        else:
                            zt = work.tile([128, TCA], BF16, tag="zt")
                            nc.scalar.activation(zt, pxz, AF.Silu)
                            nc.sync.dma_start(
                                out=d_zsp[m - 2, :, tc0:tc0 + TCA], in_=zt)
                # conv halo
                for h in range(NDH):
                    if t0 % L == 0:
                        nc.vector.memset(xpad[:, h, 0:3], 0.0)
                    else:
                        nc.vector.tensor_copy(xpad[:, h, 0:3],
                                              xpads[tcb - 1][:, h, TCC:TCC + 3])
                # conv + SiLU
                for h in range(NDH):
                    for cc in range(2):
                        off = cc * TCA
                        pc = psA.tile([128, TCA], F32, tag="ps")
                        for j in range(DC):
                            nc.tensor.matmul(pc, lhsT=cdg[:, j, h, :],
                                             rhs=xpad[:, h, off + j: off + j + TCA],
                                             start=(j == 0), stop=(j == DC - 1))
                        nc.scalar.activation(xs[:, h, t0 + off:t0 + off + TCA], pc,
                                             AF.Silu, bias=convb[:, h, 0:1], scale=1.0)
                # x_proj partials
                for cc in range(2):
                    pxp = psA.tile([96, TCA], F32, tag="ps")
                    for h in range(NDH):
                        nc.tensor.matmul(pxp, lhsT=xprj[:, h, :],
                                         rhs=xs[:, h, t0 + cc * TCA:t0 + (cc + 1) * TCA],
                                         start=(h == 0), stop=(h == NDH - 1))
                    xpt = work.tile([96, TCA], F32, tag="xpt")
                    nc.scalar.copy(xpt, pxp)
                    nc.sync.dma_start(out=d_xdp[tcb, :, cc * TCA:(cc + 1) * TCA], in_=xpt)
                nc.gpsimd.collective_compute(
                    kind="AllReduce", op=mybir.AluOpType.add, replica_groups=groups,
                    ins=[d_xdp[tcb, :, :]], outs=[d_xd[tcb, :, :]])

            nle_id = _act_table_id(nc, "natural_log_exp_and_others")

            def phaseB(tcb):
                """dt_proj + softplus for block tcb; write B/C rows to d_bc."""
                t0 = tcb * TCC
                xdt = work.tile([96, TCC], F32, tag="xdt")
                nc.sync.dma_start(out=xdt, in_=d_xd[tcb, :, :])
                xdb = work.tile([96, TCC], BF16, tag="xdb")
                nc.scalar.copy(xdb, xdt)
                nc.sync.dma_start(out=d_bc[:, t0:t0 + TCC], in_=xdb[DTR:DTR + 2 * DS, :])
                # all 4 Exps first, then all 4 Lns (avoids ACT table thrash)
                spe4 = work.tile([128, NDH, 2, TCA], BF16, tag="spe4")
                for h in range(NDH):
                    for j in range(2):
                        pdt = psA.tile([128, TCA], F32, tag="ps")
                        nc.tensor.matmul(pdt, lhsT=dtw[:, h * 128:(h + 1) * 128],
                                         rhs=xdb[0:DTR, j * TCA:(j + 1) * TCA],
                                         start=True, stop=True)
                        nc.scalar.activation(spe4[:, h, j, :], pdt, AF.Exp,
                                             bias=dtb[:, h, 0:1], scale=1.0)
                for h in range(NDH):
                    for j in range(2):
                        nc.scalar.activation(dts[:, h, t0 + j * TCA:t0 + (j + 1) * TCA],
                                             spe4[:, h, j, :], AF.Ln, bias=1.0, scale=1.0)

            def phaseC_head(tcb):
                """D-skip matmuls + dtx + first n-pairs of block tcb (emitted
                before phaseB so the scan pipeline never starves)."""
                t0 = tcb * TCC
                # pin the Exp+Ln table for this round (C exps + B softplus);
                # the auto-chooser would thrash exp_and_others<->natural_log
                nc.scalar.add_instruction(mybir.InstLoadActFuncSet(
                    name=nc.get_next_instruction_name(),
                    act_func_set_id=nle_id, ins=[], outs=[]))
                dtx2 = [None, None]
                for h in range(NDH):
                    for qq in range(2):
                        nc.tensor.matmul(pys[h][qq], lhsT=ddg[:, h, :],
                                         rhs=xs[:, h, t0 + qq * 512:t0 + (qq + 1) * 512],
                                         start=True, stop=False)
                    dtx2[h] = workC.tile([128, 2, TCC], BF16, tag="dtx2",
                                         name=f"dtx2_{tcb}_{h}")
                    nc.vector.tensor_mul(dtx2[h][:, 0, :], dts[:, h, t0:t0 + TCC],
                                         xs[:, h, t0:t0 + TCC])
                    nc.vector.tensor_copy(dtx2[h][:, 1, :], dtx2[h][:, 0, :])
                phaseC_pairs(tcb, dtx2, 0, CHEAD)
                return dtx2

            def phaseC_pairs(tcb, dtx2, p0, p1):
                t0 = tcb * TCC
                for np_ in range(p0, p1):
                    n0 = 2 * np_
                    bbc2 = bcp.tile([128, 2, TCC], BF16, tag="bbc2")
                    cbc2 = bcp.tile([128, 2, TCC], BF16, tag="cbc2")
                    for s in range(2):
                        nc.sync.dma_start(
                            out=bbc2[:, s, :],
                            in_=bass.AP(tensor=d_bc[:, :].tensor,
                                        offset=(n0 + s) * T + t0,
                                        ap=[[0, 128], [1, TCC]]))
                        nc.sync.dma_start(
                            out=cbc2[:, s, :],
                            in_=bass.AP(tensor=d_bc[:, :].tensor,
                                        offset=(DS + n0 + s) * T + t0,
                                        ap=[[0, 128], [1, TCC]]))
                    for h in range(NDH):
                        dBx2 = workC.tile([128, 2, TCC], BF16, tag="dBx2")
                        if np_ < NPP:
                            nc.gpsimd.tensor_mul(dBx2, dtx2[h], bbc2)
                        else:
                            nc.vector.tensor_mul(dBx2, dtx2[h], bbc2)
                        hts2 = workC.tile([128, 2, TCC], BF16, tag="hts2")
                        for s in range(2):
                            n = n0 + s
                            dA = workC.tile([128, TCC], BF16, tag="dA")
                            nc.scalar.activation(dA, dts[:, h, t0:t0 + TCC], AF.Exp,
                                                 bias=0.0, scale=aneg[:, h, n:n + 1])
                            init = 0.0 if (t0 % L == 0) else carry[:, h, n:n + 1]
                            nc.vector.tensor_tensor_scan(
                                out=hts2[:, s, :], data0=dA, data1=dBx2[:, s, :],
                                initial=init,
                                op0=mybir.AluOpType.mult, op1=mybir.AluOpType.add)
                        if (t0 + TCC) % L != 0:
                            nc.vector.tensor_copy(carry[:, h, n0:n0 + 2],
                                                  hts2[:, :, TCC - 1:TCC])
                        yp2 = workC.tile([128, 2, TCC], BF16, tag="yp2")
                        if np_ < NPP:
                            nc.gpsimd.tensor_mul(yp2, hts2, cbc2)
                        else:
                            nc.vector.tensor_mul(yp2, hts2, cbc2)
                        for s in range(2):
                            for qq in range(2):
                                nc.tensor.matmul(
                                    pys[h][qq], lhsT=ident,
                                    rhs=yp2[:, s, qq * 512:(qq + 1) * 512],
                                    start=False,
                                    stop=(np_ == DS // 2 - 1 and s == 1))

            def phaseC_tail(tcb, dtx2):
                """remaining n-pairs + gate + A2A input write."""
                t0 = tcb * TCC
                phaseC_pairs(tcb, dtx2, CHEAD, DS // 2)
                for h in range(NDH):
                    ybase = workC.tile([128, TCC], BF16, tag="ybase")
                    for qq in range(2):
                        nc.scalar.copy(ybase[:, qq * 512:(qq + 1) * 512], pys[h][qq])
                    zs = workC.tile([128, TCC], BF16, tag="zs")
                    nc.sync.dma_start(out=zs, in_=d_zsp[h, :, t0:t0 + TCC])
                    yg = workC.tile([128, TCC], BF16, tag="yg")
                    nc.vector.tensor_mul(yg, ybase, zs)
                    nc.sync.dma_start(out=d_a2ai[h][tcb, :, :], in_=yg)

            # ---- software pipeline over t-blocks -------------------------
            # emit order per round: C-head(r-2) (instantly-ready scan work),
            # B(r-1) (its dep chain resolves under the head), C-tail(r-2),
            # then A(r) so A's ACT/TE work queues BEHIND scan-critical ops.
            for r in range(NTCB + 2):
                dtx2 = phaseC_head(r - 2) if r >= 2 else None
                if 1 <= r <= NTCB:
                    phaseB(r - 1)
                if r >= 2:
                    phaseC_tail(r - 2, dtx2)
                if r < NTCB:
                    phaseA(r)

        # ---- AllToAll (split by channel half; E's first half overlaps
        # the second transfer) ---------------------------------------------
        for h in range(NDH):
            nc.gpsimd.collective_compute(
                kind="AllToAll", op=mybir.AluOpType.bypass, replica_groups=groups,
                ins=[d_a2ai[h][:, :, :]], outs=[d_a2ao[h][:, :, :]])

        # ---- out_proj: 2 dm-half passes, 8 PSUM accumulators -------------
        with tc.tile_pool(name="workE", bufs=2) as workE, \
             tc.tile_pool(name="psE", bufs=1, space="PSUM") as psE:
            pos = [psE.tile([128, 512], F32, tag=f"pe_{ts}", name=f"pe_{ts}")
                   for ts in range(8)]
            # keep the PE clock ramped through the AllToAll so out_proj
            # runs at full rate (HAM drops the clock after ~3.4us idle)
            for w in range(96):
                nc.tensor.matmul(pos[w % 8], lhsT=ident, rhs=wxz[:, 0, 0:512],
                                 start=True, stop=True)
            for fh in range(2):
                for kt in range(2 * NCORES):
                    h, i = kt // NCORES, kt % NCORES
                    ykt = workE.tile([128, TSL], BF16, tag="ykS")
                    nc.sync.dma_start(out=ykt, in_=d_a2ao[h][i, :, :])
                    wot = workE.tile([128, 512], BF16, tag="woS")
                    nc.sync.dma_start(
                        out=wot, in_=d_woutT[2 * i + h, :, fh * 512:(fh + 1) * 512])
                    for ts in range(8):
                        nc.tensor.matmul(pos[ts], lhsT=ykt[:, ts * 128:(ts + 1) * 128],
                                         rhs=wot, start=(kt == 0), stop=(kt == 15))
                for ts in range(8):
                    ot = workE.tile([128, 512], F32, tag="otS")
                    nc.scalar.copy(ot, pos[ts])
                    nc.sync.dma_start(
                        out=d_out[ts * 128:(ts + 1) * 128, fh * 512:(fh + 1) * 512],
                        in_=ot)

    nc.compile()
    return nc


def _host_prep(inputs):
    """Per-core input maps from full inputs (layout prep + bf16 casts only)."""
    hs = np.asarray(inputs["hidden_states"], np.float32)
    wxz = np.asarray(inputs["in_proj_w"], np.float32)
    cw = np.asarray(inputs["conv_w"], np.float32)
    cb = np.asarray(inputs["conv_b"], np.float32)
    xpw = np.asarray(inputs["x_proj_w"], np.float32)
    dpw = np.asarray(inputs["dt_proj_w"], np.float32)
    dpb = np.asarray(inputs["dt_proj_b"], np.float32)
    alog = np.asarray(inputs["A_log"], np.float32)
    dv = np.asarray(inputs["D"], np.float32)
    wo = np.asarray(inputs["out_proj_w"], np.float32)

    hT = np.ascontiguousarray(hs.reshape(T, DM).T).astype(BF)
    woutT = np.ascontiguousarray(wo.T).reshape(2 * NCORES, 128, DM).astype(BF)
    ident = np.eye(128, dtype=np.float32).astype(BF)

    in_maps = []
    for i in range(NCORES):
        lo = i * DL
        sl = slice(lo, lo + DL)
        wxzT = np.ascontiguousarray(
            np.concatenate([wxz[sl], wxz[DI + lo:DI + lo + DL]], axis=0).T).astype(BF)
        cdiag = np.zeros((DC, NDH, 128, 128), np.float32)
        for j in range(DC):
            for h in range(NDH):
                np.fill_diagonal(cdiag[j, h], cw[lo + h * 128:lo + (h + 1) * 128, j])
        ddiag = np.zeros((NDH, 128, 128), np.float32)
        for h in range(NDH):
            np.fill_diagonal(ddiag[h], dv[lo + h * 128:lo + (h + 1) * 128])
        in_maps.append({
            "hT": hT,
            "wxzT": wxzT,
            "cdiag": cdiag.astype(BF),
            "convb": cb[sl].reshape(NDH, 128, 1),
            "xprojT": np.ascontiguousarray(xpw[:, sl].T).reshape(NDH, 128, 96).astype(BF),
            "dtwT": np.ascontiguousarray(dpw[sl].T).astype(BF),
            "dtb": dpb[sl].reshape(NDH, 128, 1),
            "aneg": (-np.exp(alog[sl])).reshape(NDH, 128, DS).astype(np.float32),
            "ddiag": ddiag.astype(BF),
            "woutT": woutT,
            "ident": ident,
        })
    return in_maps


def _run(inputs, trace=False, **kw):
    if "nc" not in _cached:
        _cached["nc"] = _build()
    nc = _cached["nc"]
    in_maps = _host_prep(inputs)
    res = bass_utils.run_bass_kernel_spmd(
        nc, in_maps, core_ids=list(range(NCORES)), trace=trace, **kw)
    out = np.concatenate([res.results[i]["out_slice"] for i in range(NCORES)], axis=0)
    return out.reshape(B, L, DM).astype(np.float32), res


def kernel(**inputs):
    out, _ = _run(inputs, trace=False)
    return out


# revision 41
# speedup vs baseline: 1.1433x; 1.1433x over previous
"""Trainium2 Bass kernel for a dense Mamba (selective-scan) block, SPMD over 8 NeuronCores.

v3: tensor-parallel over d_inner (2048 -> 256 ch/core), software-pipelined at
t-block (1024) granularity: round r runs in_proj/conv/x_proj + per-block
AllReduce for block r while the DVE-bound scan runs block r-2. Per core:
in_proj -> z-SiLU (native Silu) -> causal conv (4 diagonal matmuls) -> SiLU ->
x_proj partial -> AllReduce (8x 384KB) -> dt softplus (Exp/Ln) -> scan per
(block, n-pair, h): dA=Exp(dts*A_n) f32, dBx pair-batched bf16 mul, hardware
tensor_tensor_scan (fp32 state), y=h*C pair-batched, n-reduction + D-skip via
PSUM-accumulated matmuls (identity + diag(D)) -> gate -> AllToAll -> out_proj
(2 dm-half passes, 8 PSUM accumulators) -> per-core t-slice, host concat.

Shapes hardcoded for: B=2, L=4096, d_model=1024, d_inner=2048, d_state=16,
d_conv=4, dt_rank=64, f32 I/O.
"""
import numpy as np
import ml_dtypes
from contextlib import ExitStack

import concourse.bass as bass
import concourse.bacc as bacc
import concourse.tile as tile
from concourse import mybir
from concourse import bass_utils

BF = ml_dtypes.bfloat16
F32 = mybir.dt.float32
BF16 = mybir.dt.bfloat16

NCORES = 8
B, L, DM = 2, 4096, 1024
DI, DS, DC, DTR = 2048, 16, 4, 64
DL = DI // NCORES          # 256 local channels
NDH = DL // 128            # 2 d-half tiles
T = B * L                  # 8192 flattened (b, l)
TSL = T // NCORES          # 1024 t-slice per core for the output
TCA = 512                  # phase A matmul chunk
TCC = 1024                 # pipeline block (== TSL)
NTCB = T // TCC            # 8
NPP = 0                    # n-pairs per (block, h) whose muls go to gpsimd
CHEAD = 2                  # n-pairs emitted before phaseB each round

AF = mybir.ActivationFunctionType

_cached = {}


def _act_table_id(nc, name):
    from concourse.hw_specs import get_activation_tables
    return list(get_activation_tables(nc.m.arch).keys()).index(name)


def _build():
    nc = bacc.Bacc("TRN2", target_bir_lowering=False, num_devices=NCORES)

    # ---- I/O -------------------------------------------------------------
    d_hT = nc.dram_tensor("hT", (DM, T), BF16, kind="ExternalInput")
    d_wxzT = nc.dram_tensor("wxzT", (DM, 2 * DL), BF16, kind="ExternalInput")
    d_cdiag = nc.dram_tensor("cdiag", (DC, NDH, 128, 128), BF16, kind="ExternalInput")
    d_convb = nc.dram_tensor("convb", (NDH, 128, 1), F32, kind="ExternalInput")
    d_xprojT = nc.dram_tensor("xprojT", (NDH, 128, DTR + 2 * DS), BF16, kind="ExternalInput")
    d_dtwT = nc.dram_tensor("dtwT", (DTR, DL), BF16, kind="ExternalInput")
    d_dtb = nc.dram_tensor("dtb", (NDH, 128, 1), F32, kind="ExternalInput")
    d_aneg = nc.dram_tensor("aneg", (NDH, 128, DS), F32, kind="ExternalInput")
    d_ddiag = nc.dram_tensor("ddiag", (NDH, 128, 128), BF16, kind="ExternalInput")
    d_woutT = nc.dram_tensor("woutT", (2 * NCORES, 128, DM), BF16, kind="ExternalInput")
    d_ident = nc.dram_tensor("ident", (128, 128), BF16, kind="ExternalInput")
    d_out = nc.dram_tensor("out_slice", (TSL, DM), F32, kind="ExternalOutput")

    # ---- internal DRAM ---------------------------------------------------
    d_xdp = nc.dram_tensor("xdp", (NTCB, DTR + 2 * DS, TCC), F32, kind="Internal")
    d_xd = nc.dram_tensor("xd", (NTCB, DTR + 2 * DS, TCC), F32, kind="Internal",
                          addr_space="Shared")
    d_zsp = nc.dram_tensor("zsp", (NDH, 128, T), BF16, kind="Internal")
    d_bc = nc.dram_tensor("bcrows", (2 * DS, T), BF16, kind="Internal")
    d_a2ai = [nc.dram_tensor(f"a2ai{h}", (NCORES, 128, TSL), BF16, kind="Internal")
              for h in range(NDH)]
    d_a2ao = [nc.dram_tensor(f"a2ao{h}", (NCORES, 128, TSL), BF16, kind="Internal")
              for h in range(NDH)]

    groups = [list(range(NCORES))]

    with tile.TileContext(nc) as tc, ExitStack() as ctx:
        consts = ctx.enter_context(tc.tile_pool(name="consts", bufs=1))
        resid = ctx.enter_context(tc.tile_pool(name="resid", bufs=1))

        # ---- load constants ----------------------------------------------
        wxz = consts.tile([128, 8, 2 * DL], BF16, tag="wxz")
        nc.sync.dma_start(out=wxz, in_=d_wxzT[:, :].rearrange("(k p) m -> p k m", p=128))
        cdg = consts.tile([128, DC, NDH, 128], BF16, tag="cdg")
        nc.sync.dma_start(
            out=cdg, in_=bass.AP(tensor=d_cdiag[:, :, :, :].tensor, offset=0,
                                 ap=[[128, 128], [NDH * 128 * 128, DC], [128 * 128, NDH], [1, 128]]))
        convb = consts.tile([128, NDH, 1], F32, tag="convb")
        nc.sync.dma_start(out=convb, in_=d_convb[:, :, :].rearrange("h p one -> p h one"))
        xprj = consts.tile([128, NDH, DTR + 2 * DS], BF16, tag="xprj")
        nc.sync.dma_start(out=xprj, in_=d_xprojT[:, :, :].rearrange("h p m -> p h m"))
        dtw = consts.tile([DTR, DL], BF16, tag="dtw")
        nc.sync.dma_start(out=dtw, in_=d_dtwT[:, :])
        dtb = consts.tile([128, NDH, 1], F32, tag="dtb")
        nc.sync.dma_start(out=dtb, in_=d_dtb[:, :, :].rearrange("h p one -> p h one"))
        aneg = consts.tile([128, NDH, DS], F32, tag="aneg")
        nc.sync.dma_start(out=aneg, in_=d_aneg[:, :, :].rearrange("h p n -> p h n"))
        ddg = consts.tile([128, NDH, 128], BF16, tag="ddg")
        nc.sync.dma_start(out=ddg, in_=d_ddiag[:, :, :].rearrange("h p q -> p h q"))
        ident = consts.tile([128, 128], BF16, tag="ident")
        nc.sync.dma_start(out=ident, in_=d_ident[:, :])
        carry = consts.tile([128, NDH, DS], F32, tag="carry")

        # ---- SBUF-resident activations -----------------------------------
        xs = resid.tile([128, NDH, T], BF16, tag="xs")        # silu(conv(x))
        dts = resid.tile([128, NDH, T], BF16, tag="dts")      # softplus dt

        with tc.tile_pool(name="xpadp", bufs=2) as xpadp, \
             tc.tile_pool(name="work", bufs=2) as work, \
             tc.tile_pool(name="workC", bufs=2) as workC, \
             tc.tile_pool(name="bcp", bufs=3) as bcp, \
             tc.tile_pool(name="dAp", bufs=4) as dAp, \
             tc.tile_pool(name="psA", bufs=3, space="PSUM") as psA, \
             tc.tile_pool(name="psY", bufs=1, space="PSUM") as psY:

            pys = [[psY.tile([128, 512], F32, tag=f"py_{h}_{qq}", name=f"py_{h}_{qq}")
                    for qq in range(2)] for h in range(NDH)]
            xpads = {}

            def phaseA(tcb):
                """in_proj + conv + x_proj partial + AllReduce for block tcb."""
                t0 = tcb * TCC
                xpad = xpadp.tile([128, NDH, 3 + TCC], BF16, tag="xpad",
                                  name=f"xpad{tcb}")
                xpads[tcb] = xpad
                # in_proj: 2 chunks of 512
                for cc in range(2):
                    tc0 = t0 + cc * TCA
                    ht = work.tile([128, 8, TCA], BF16, tag="ht")
                    nc.sync.dma_start(
                        out=ht,
                        in_=bass.AP(tensor=d_hT[:, :].tensor, offset=tc0,
                                    ap=[[T, 128], [128 * T, 8], [1, TCA]]))
                    for m in range(4):  # 0,1: x halves; 2,3: z halves
                        pxz = psA.tile([128, TCA], F32, tag="ps")
                        for k in range(8):
                            nc.tensor.matmul(pxz, lhsT=wxz[:, k, m * 128:(m + 1) * 128],
                                             rhs=ht[:, k, :], start=(k == 0), stop=(k == 7))
                        if m < 2:
                            nc.scalar.copy(
                                xpad[:, m, 3 + cc * TCA: 3 + (cc + 1) * TCA], pxz)
                        else:
                            zt = work.tile([128, TCA], BF16, tag="zt")
                            nc.scalar.activation(zt, pxz, AF.Silu)
                            nc.sync.dma_start(
                                out=d_zsp[m - 2, :, tc0:tc0 + TCA], in_=zt)
                # conv halo
                for h in range(NDH):
                    if t0 % L == 0:
                        nc.vector.memset(xpad[:, h, 0:3], 0.0)
                    else:
                        nc.vector.tensor_copy(xpad[:, h, 0:3],
                                              xpads[tcb - 1][:, h, TCC:TCC + 3])
                # conv + SiLU
                for h in range(NDH):
                    for cc in range(2):
                        off = cc * TCA
                        pc = psA.tile([128, TCA], F32, tag="ps")
                        for j in range(DC):
                            nc.tensor.matmul(pc, lhsT=cdg[:, j, h, :],
                                             rhs=xpad[:, h, off + j: off + j + TCA],
                                             start=(j == 0), stop=(j == DC - 1))
                        nc.scalar.activation(xs[:, h, t0 + off:t0 + off + TCA], pc,
                                             AF.Silu, bias=convb[:, h, 0:1], scale=1.0)
                # x_proj partials
                for cc in range(2):
                    pxp = psA.tile([96, TCA], F32, tag="ps")
                    for h in range(NDH):
                        nc.tensor.matmul(pxp, lhsT=xprj[:, h, :],
                                         rhs=xs[:, h, t0 + cc * TCA:t0 + (cc + 1) * TCA],
                                         start=(h == 0), stop=(h == NDH - 1))
                    xpt = work.tile([96, TCA], F32, tag="xpt")
                    nc.scalar.copy(xpt, pxp)
                    nc.sync.dma_start(out=d_xdp[tcb, :, cc * TCA:(cc + 1) * TCA], in_=xpt)
                nc.gpsimd.collective_compute(
                    kind="AllReduce", op=mybir.AluOpType.add, replica_groups=groups,
                    ins=[d_xdp[tcb, :, :]], outs=[d_xd[tcb, :, :]])

            nle_id = _act_table_id(nc, "natural_log_exp_and_others")

            def phaseB(tcb):
                """dt_proj + softplus for block tcb; write B/C rows to d_bc."""
                t0 = tcb * TCC
                xdt = work.tile([96, TCC], F32, tag="xdt")
                nc.sync.dma_start(out=xdt, in_=d_xd[tcb, :, :])
                xdb = work.tile([96, TCC], BF16, tag="xdb")
                nc.scalar.copy(xdb, xdt)
                nc.sync.dma_start(out=d_bc[:, t0:t0 + TCC], in_=xdb[DTR:DTR + 2 * DS, :])
                # all 4 Exps first, then all 4 Lns (avoids ACT table thrash)
                spe4 = work.tile([128, NDH, 2, TCA], BF16, tag="spe4")
                for h in range(NDH):
                    for j in range(2):
                        pdt = psA.tile([128, TCA], F32, tag="ps")
                        nc.tensor.matmul(pdt, lhsT=dtw[:, h * 128:(h + 1) * 128],
                                         rhs=xdb[0:DTR, j * TCA:(j + 1) * TCA],
                                         start=True, stop=True)
                        nc.scalar.activation(spe4[:, h, j, :], pdt, AF.Exp,
                                             bias=dtb[:, h, 0:1], scale=1.0)
                for h in range(NDH):
                    for j in range(2):
                        nc.scalar.activation(dts[:, h, t0 + j * TCA:t0 + (j + 1) * TCA],
                                             spe4[:, h, j, :], AF.Ln, bias=1.0, scale=1.0)

            def phaseC_head(tcb):
                """D-skip matmuls + dtx + first n-pairs of block tcb (emitted
                before phaseB so the scan pipeline never starves)."""
                t0 = tcb * TCC
                # pin the Exp+Ln table for this round (C exps + B softplus);
                # the auto-chooser would thrash exp_and_others<->natural_log
                nc.scalar.add_instruction(mybir.InstLoadActFuncSet(
                    name=nc.get_next_instruction_name(),
                    act_func_set_id=nle_id, ins=[], outs=[]))
                dtx2 = [None, None]
                for h in range(NDH):
                    for qq in range(2):
                        nc.tensor.matmul(pys[h][qq], lhsT=ddg[:, h, :],
                                         rhs=xs[:, h, t0 + qq * 512:t0 + (qq + 1) * 512],
                                         start=True, stop=False)
                    dtx2[h] = workC.tile([128, 2, TCC], BF16, tag="dtx2",
                                         name=f"dtx2_{tcb}_{h}")
                    nc.vector.tensor_mul(dtx2[h][:, 0, :], dts[:, h, t0:t0 + TCC],
                                         xs[:, h, t0:t0 + TCC])
                    nc.vector.tensor_copy(dtx2[h][:, 1, :], dtx2[h][:, 0, :])
                phaseC_pairs(tcb, dtx2, 0, CHEAD)
                return dtx2

            def phaseC_pairs(tcb, dtx2, p0, p1):
                t0 = tcb * TCC
                for np_ in range(p0, p1):
                    n0 = 2 * np_
                    bbc2 = bcp.tile([128, 2, TCC], BF16, tag="bbc2")
                    cbc2 = bcp.tile([128, 2, TCC], BF16, tag="cbc2")
                    for s in range(2):
                        nc.sync.dma_start(
                            out=bbc2[:, s, :],
                            in_=bass.AP(tensor=d_bc[:, :].tensor,
                                        offset=(n0 + s) * T + t0,
                                        ap=[[0, 128], [1, TCC]]))
                        nc.sync.dma_start(
                            out=cbc2[:, s, :],
                            in_=bass.AP(tensor=d_bc[:, :].tensor,
                                        offset=(DS + n0 + s) * T + t0,
                                        ap=[[0, 128], [1, TCC]]))
                    for h in range(NDH):
                        dBx2 = workC.tile([128, 2, TCC], BF16, tag="dBx2")
                        if np_ < NPP:
                            nc.gpsimd.tensor_mul(dBx2, dtx2[h], bbc2)
                        else:
                            nc.vector.tensor_mul(dBx2, dtx2[h], bbc2)
                        hts2 = workC.tile([128, 2, TCC], BF16, tag="hts2")
                        for s in range(2):
                            n = n0 + s
                            dA = dAp.tile([128, TCC], BF16, tag="dA")
                            nc.scalar.activation(dA, dts[:, h, t0:t0 + TCC], AF.Exp,
                                                 bias=0.0, scale=aneg[:, h, n:n + 1])
                            init = 0.0 if (t0 % L == 0) else carry[:, h, n:n + 1]
                            nc.vector.tensor_tensor_scan(
                                out=hts2[:, s, :], data0=dA, data1=dBx2[:, s, :],
                                initial=init,
                                op0=mybir.AluOpType.mult, op1=mybir.AluOpType.add)
                        if (t0 + TCC) % L != 0:
                            nc.vector.tensor_copy(carry[:, h, n0:n0 + 2],
                                                  hts2[:, :, TCC - 1:TCC])
                        yp2 = workC.tile([128, 2, TCC], BF16, tag="yp2")
                        if np_ < NPP:
                            nc.gpsimd.tensor_mul(yp2, hts2, cbc2)
                        else:
                            nc.vector.tensor_mul(yp2, hts2, cbc2)
                        for s in range(2):
                            for qq in range(2):
                                nc.tensor.matmul(
                                    pys[h][qq], lhsT=ident,
                                    rhs=yp2[:, s, qq * 512:(qq + 1) * 512],
                                    start=False,
                                    stop=(np_ == DS // 2 - 1 and s == 1))

            def phaseC_tail(tcb, dtx2):
                """remaining n-pairs + gate + A2A input write."""
                t0 = tcb * TCC
                phaseC_pairs(tcb, dtx2, CHEAD, DS // 2)
                for h in range(NDH):
                    ybase = workC.tile([128, TCC], BF16, tag="ybase")
                    for qq in range(2):
                        nc.scalar.copy(ybase[:, qq * 512:(qq + 1) * 512], pys[h][qq])
                    zs = workC.tile([128, TCC], BF16, tag="zs")
                    nc.sync.dma_start(out=zs, in_=d_zsp[h, :, t0:t0 + TCC])
                    yg = workC.tile([128, TCC], BF16, tag="yg")
                    nc.vector.tensor_mul(yg, ybase, zs)
                    nc.sync.dma_start(out=d_a2ai[h][tcb, :, :], in_=yg)

            # ---- software pipeline over t-blocks -------------------------
            # emit order per round: C-head(r-2) (instantly-ready scan work),
            # B(r-1) (its dep chain resolves under the head), C-tail(r-2),
            # then A(r) so A's ACT/TE work queues BEHIND scan-critical ops.
            for r in range(NTCB + 2):
                dtx2 = phaseC_head(r - 2) if r >= 2 else None
                if 1 <= r <= NTCB:
                    phaseB(r - 1)
                if r >= 2:
                    phaseC_tail(r - 2, dtx2)
                if r < NTCB:
                    phaseA(r)

        # ---- AllToAll (split by channel half; E's first half overlaps
        # the second transfer) ---------------------------------------------
        for h in range(NDH):
            nc.gpsimd.collective_compute(
                kind="AllToAll", op=mybir.AluOpType.bypass, replica_groups=groups,
                ins=[d_a2ai[h][:, :, :]], outs=[d_a2ao[h][:, :, :]])

        # ---- out_proj: 2 dm-half passes, 8 PSUM accumulators -------------
        with tc.tile_pool(name="workE", bufs=2) as workE, \
             tc.tile_pool(name="psE", bufs=1, space="PSUM") as psE:
            pos = [psE.tile([128, 512], F32, tag=f"pe_{ts}", name=f"pe_{ts}")
                   for ts in range(8)]
            for fh in range(2):
                for kt in range(2 * NCORES):
                    h, i = kt // NCORES, kt % NCORES
                    ykt = workE.tile([128, TSL], BF16, tag="ykS")
                    nc.sync.dma_start(out=ykt, in_=d_a2ao[h][i, :, :])
                    wot = workE.tile([128, 512], BF16, tag="woS")
                    nc.sync.dma_start(
                        out=wot, in_=d_woutT[2 * i + h, :, fh * 512:(fh + 1) * 512])
                    for ts in range(8):
                        nc.tensor.matmul(pos[ts], lhsT=ykt[:, ts * 128:(ts + 1) * 128],
                                         rhs=wot, start=(kt == 0), stop=(kt == 15))
                for ts in range(8):
                    ot = workE.tile([128, 512], F32, tag="otS")
                    nc.scalar.copy(ot, pos[ts])
                    nc.sync.dma_start(
                        out=d_out[ts * 128:(ts + 1) * 128, fh * 512:(fh + 1) * 512],
                        in_=ot)

    nc.compile()
    return nc


def _host_prep(inputs):
    """Per-core input maps from full inputs (layout prep + bf16 casts only)."""
    hs = np.asarray(inputs["hidden_states"], np.float32)
    wxz = np.asarray(inputs["in_proj_w"], np.float32)
    cw = np.asarray(inputs["conv_w"], np.float32)
    cb = np.asarray(inputs["conv_b"], np.float32)
    xpw = np.asarray(inputs["x_proj_w"], np.float32)
    dpw = np.asarray(inputs["dt_proj_w"], np.float32)
    dpb = np.asarray(inputs["dt_proj_b"], np.float32)
    alog = np.asarray(inputs["A_log"], np.float32)
    dv = np.asarray(inputs["D"], np.float32)
    wo = np.asarray(inputs["out_proj_w"], np.float32)

    hT = np.ascontiguousarray(hs.reshape(T, DM).T).astype(BF)
    woutT = np.ascontiguousarray(wo.T).reshape(2 * NCORES, 128, DM).astype(BF)
    ident = np.eye(128, dtype=np.float32).astype(BF)

    in_maps = []
    for i in range(NCORES):
        lo = i * DL
        sl = slice(lo, lo + DL)
        wxzT = np.ascontiguousarray(
            np.concatenate([wxz[sl], wxz[DI + lo:DI + lo + DL]], axis=0).T).astype(BF)
        cdiag = np.zeros((DC, NDH, 128, 128), np.float32)
        for j in range(DC):
            for h in range(NDH):
                np.fill_diagonal(cdiag[j, h], cw[lo + h * 128:lo + (h + 1) * 128, j])
        ddiag = np.zeros((NDH, 128, 128), np.float32)
        for h in range(NDH):
            np.fill_diagonal(ddiag[h], dv[lo + h * 128:lo + (h + 1) * 128])
        in_maps.append({
            "hT": hT,
            "wxzT": wxzT,
            "cdiag": cdiag.astype(BF),
            "convb": cb[sl].reshape(NDH, 128, 1),
            "xprojT": np.ascontiguousarray(xpw[:, sl].T).reshape(NDH, 128, 96).astype(BF),
            "dtwT": np.ascontiguousarray(dpw[sl].T).astype(BF),
            "dtb": dpb[sl].reshape(NDH, 128, 1),
            "aneg": (-np.exp(alog[sl])).reshape(NDH, 128, DS).astype(np.float32),
            "ddiag": ddiag.astype(BF),
            "woutT": woutT,
            "ident": ident,
        })
    return in_maps


def _run(inputs, trace=False, **kw):
    if "nc" not in _cached:
        _cached["nc"] = _build()
    nc = _cached["nc"]
    in_maps = _host_prep(inputs)
    res = bass_utils.run_bass_kernel_spmd(
        nc, in_maps, core_ids=list(range(NCORES)), trace=trace, **kw)
    out = np.concatenate([res.results[i]["out_slice"] for i in range(NCORES)], axis=0)
    return out.reshape(B, L, DM).astype(np.float32), res


def kernel(**inputs):
    out, _ = _run(inputs, trace=False)
    return out


# revision 43
# speedup vs baseline: 1.1773x; 1.0297x over previous
"""Trainium2 Bass kernel for a dense Mamba (selective-scan) block, SPMD over 8 NeuronCores.

v3: tensor-parallel over d_inner (2048 -> 256 ch/core), software-pipelined at
t-block (1024) granularity: round r runs in_proj/conv/x_proj + per-block
AllReduce for block r while the DVE-bound scan runs block r-2. Per core:
in_proj -> z-SiLU (native Silu) -> causal conv (4 diagonal matmuls) -> SiLU ->
x_proj partial -> AllReduce (8x 384KB) -> dt softplus (Exp/Ln) -> scan per
(block, n-pair, h): dA=Exp(dts*A_n) f32, dBx pair-batched bf16 mul, hardware
tensor_tensor_scan (fp32 state), y=h*C pair-batched, n-reduction + D-skip via
PSUM-accumulated matmuls (identity + diag(D)) -> gate -> AllToAll -> out_proj
(2 dm-half passes, 8 PSUM accumulators) -> per-core t-slice, host concat.

Shapes hardcoded for: B=2, L=4096, d_model=1024, d_inner=2048, d_state=16,
d_conv=4, dt_rank=64, f32 I/O.
"""
import numpy as np
import ml_dtypes
from contextlib import ExitStack

import concourse.bass as bass
import concourse.bacc as bacc
import concourse.tile as tile
from concourse import mybir
from concourse import bass_utils

BF = ml_dtypes.bfloat16
F32 = mybir.dt.float32
BF16 = mybir.dt.bfloat16

NCORES = 8
B, L, DM = 2, 4096, 1024
DI, DS, DC, DTR = 2048, 16, 4, 64
DL = DI // NCORES          # 256 local channels
NDH = DL // 128            # 2 d-half tiles
T = B * L                  # 8192 flattened (b, l)
TSL = T // NCORES          # 1024 t-slice per core for the output
TCA = 512                  # phase A matmul chunk
TCC = 1024                 # pipeline block (== TSL)
NTCB = T // TCC            # 8
NPP = 0                    # n-pairs per (block, h) whose muls go to gpsimd
CHEAD = 2                  # n-pairs emitted before phaseB each round

AF = mybir.ActivationFunctionType

_cached = {}


def _act_table_id(nc, name):
    from concourse.hw_specs import get_activation_tables
    return list(get_activation_tables(nc.m.arch).keys()).index(name)


def _build():
    nc = bacc.Bacc("TRN2", target_bir_lowering=False, num_devices=NCORES)

    # ---- I/O -------------------------------------------------------------
    d_hT = nc.dram_tensor("hT", (DM, T), BF16, kind="ExternalInput")
    d_wxzT = nc.dram_tensor("wxzT", (DM, 2 * DL), BF16, kind="ExternalInput")
    d_cdiag = nc.dram_tensor("cdiag", (DC, NDH, 128, 128), BF16, kind="ExternalInput")
    d_convb = nc.dram_tensor("convb", (NDH, 128, 1), F32, kind="ExternalInput")
    d_xprojT = nc.dram_tensor("xprojT", (NDH, 128, DTR + 2 * DS), BF16, kind="ExternalInput")
    d_dtwT = nc.dram_tensor("dtwT", (DTR, DL), BF16, kind="ExternalInput")
    d_dtb = nc.dram_tensor("dtb", (NDH, 128, 1), F32, kind="ExternalInput")
    d_aneg = nc.dram_tensor("aneg", (NDH, 128, DS), F32, kind="ExternalInput")
    d_ddiag = nc.dram_tensor("ddiag", (NDH, 128, 128), BF16, kind="ExternalInput")
    d_woutT = nc.dram_tensor("woutT", (2 * NCORES, 128, DM), BF16, kind="ExternalInput")
    d_ident = nc.dram_tensor("ident", (128, 128), BF16, kind="ExternalInput")
    d_out = nc.dram_tensor("out_slice", (TSL, DM), F32, kind="ExternalOutput")

    # ---- internal DRAM ---------------------------------------------------
    d_xdp = nc.dram_tensor("xdp", (NTCB, DTR + 2 * DS, TCC), F32, kind="Internal")
    d_xd = nc.dram_tensor("xd", (NTCB, DTR + 2 * DS, TCC), F32, kind="Internal",
                          addr_space="Shared")
    d_zsp = nc.dram_tensor("zsp", (NDH, 128, T), BF16, kind="Internal")
    d_bc = nc.dram_tensor("bcrows", (2 * DS, T), BF16, kind="Internal")
    d_a2ai = [nc.dram_tensor(f"a2ai{h}", (NCORES, 128, TSL), BF16, kind="Internal")
              for h in range(NDH)]
    d_a2ao = [nc.dram_tensor(f"a2ao{h}", (NCORES, 128, TSL), BF16, kind="Internal")
              for h in range(NDH)]

    groups = [list(range(NCORES))]

    with tile.TileContext(nc) as tc, ExitStack() as ctx:
        consts = ctx.enter_context(tc.tile_pool(name="consts", bufs=1))
        resid = ctx.enter_context(tc.tile_pool(name="resid", bufs=1))

        # ---- load constants ----------------------------------------------
        wxz = consts.tile([128, 8, 2 * DL], BF16, tag="wxz")
        nc.sync.dma_start(out=wxz, in_=d_wxzT[:, :].rearrange("(k p) m -> p k m", p=128))
        cdg = consts.tile([128, DC, NDH, 128], BF16, tag="cdg")
        nc.sync.dma_start(
            out=cdg, in_=bass.AP(tensor=d_cdiag[:, :, :, :].tensor, offset=0,
                                 ap=[[128, 128], [NDH * 128 * 128, DC], [128 * 128, NDH], [1, 128]]))
        convb = consts.tile([128, NDH, 1], F32, tag="convb")
        nc.sync.dma_start(out=convb, in_=d_convb[:, :, :].rearrange("h p one -> p h one"))
        xprj = consts.tile([128, NDH, DTR + 2 * DS], BF16, tag="xprj")
        nc.sync.dma_start(out=xprj, in_=d_xprojT[:, :, :].rearrange("h p m -> p h m"))
        dtw = consts.tile([DTR, DL], BF16, tag="dtw")
        nc.sync.dma_start(out=dtw, in_=d_dtwT[:, :])
        dtb = consts.tile([128, NDH, 1], F32, tag="dtb")
        nc.sync.dma_start(out=dtb, in_=d_dtb[:, :, :].rearrange("h p one -> p h one"))
        aneg = consts.tile([128, NDH, DS], F32, tag="aneg")
        nc.sync.dma_start(out=aneg, in_=d_aneg[:, :, :].rearrange("h p n -> p h n"))
        ddg = consts.tile([128, NDH, 128], BF16, tag="ddg")
        nc.sync.dma_start(out=ddg, in_=d_ddiag[:, :, :].rearrange("h p q -> p h q"))
        ident = consts.tile([128, 128], BF16, tag="ident")
        nc.sync.dma_start(out=ident, in_=d_ident[:, :])
        carry = consts.tile([128, NDH, DS], F32, tag="carry")

        # ---- SBUF-resident activations -----------------------------------
        xs = resid.tile([128, NDH, T], BF16, tag="xs")        # silu(conv(x))
        dts = resid.tile([128, NDH, T], BF16, tag="dts")      # softplus dt

        with tc.tile_pool(name="xpadp", bufs=2) as xpadp, \
             tc.tile_pool(name="work", bufs=2) as work, \
             tc.tile_pool(name="workC", bufs=2) as workC, \
             tc.tile_pool(name="bcp", bufs=3) as bcp, \
             tc.tile_pool(name="dAp", bufs=4) as dAp, \
             tc.tile_pool(name="psA", bufs=3, space="PSUM") as psA, \
             tc.tile_pool(name="psY", bufs=1, space="PSUM") as psY:

            pys = [[psY.tile([128, 512], F32, tag=f"py_{h}_{qq}", name=f"py_{h}_{qq}")
                    for qq in range(2)] for h in range(NDH)]
            xpads = {}

            def phaseA(tcb):
                """in_proj + conv + x_proj partial + AllReduce for block tcb."""
                t0 = tcb * TCC
                xpad = xpadp.tile([128, NDH, 3 + TCC], BF16, tag="xpad",
                                  name=f"xpad{tcb}")
                xpads[tcb] = xpad
                # in_proj: 2 chunks of 512
                for cc in range(2):
                    tc0 = t0 + cc * TCA
                    ht = work.tile([128, 8, TCA], BF16, tag="ht")
                    nc.sync.dma_start(
                        out=ht,
                        in_=bass.AP(tensor=d_hT[:, :].tensor, offset=tc0,
                                    ap=[[T, 128], [128 * T, 8], [1, TCA]]))
                    for m in range(4):  # 0,1: x halves; 2,3: z halves
                        pxz = psA.tile([128, TCA], F32, tag="ps")
                        for k in range(8):
                            nc.tensor.matmul(pxz, lhsT=wxz[:, k, m * 128:(m + 1) * 128],
                                             rhs=ht[:, k, :], start=(k == 0), stop=(k == 7))
                        if m < 2:
                            nc.scalar.copy(
                                xpad[:, m, 3 + cc * TCA: 3 + (cc + 1) * TCA], pxz)
                        else:
                            zt = work.tile([128, TCA], BF16, tag="zt")
                            nc.scalar.activation(zt, pxz, AF.Silu)
                            nc.sync.dma_start(
                                out=d_zsp[m - 2, :, tc0:tc0 + TCA], in_=zt)
                # conv halo
                for h in range(NDH):
                    if t0 % L == 0:
                        nc.vector.memset(xpad[:, h, 0:3], 0.0)
                    else:
                        nc.vector.tensor_copy(xpad[:, h, 0:3],
                                              xpads[tcb - 1][:, h, TCC:TCC + 3])
                # conv + SiLU + x_proj partial, per 512-chunk (so the
                # AllReduce input is ready as early as possible)
                for cc in range(2):
                    off = cc * TCA
                    for h in range(NDH):
                        pc = psA.tile([128, TCA], F32, tag="ps")
                        for j in range(DC):
                            nc.tensor.matmul(pc, lhsT=cdg[:, j, h, :],
                                             rhs=xpad[:, h, off + j: off + j + TCA],
                                             start=(j == 0), stop=(j == DC - 1))
                        nc.scalar.activation(xs[:, h, t0 + off:t0 + off + TCA], pc,
                                             AF.Silu, bias=convb[:, h, 0:1], scale=1.0)
                    pxp = psA.tile([96, TCA], F32, tag="ps")
                    for h in range(NDH):
                        nc.tensor.matmul(pxp, lhsT=xprj[:, h, :],
                                         rhs=xs[:, h, t0 + off:t0 + off + TCA],
                                         start=(h == 0), stop=(h == NDH - 1))
                    xpt = work.tile([96, TCA], F32, tag="xpt")
                    nc.scalar.copy(xpt, pxp)
                    nc.sync.dma_start(out=d_xdp[tcb, :, off:off + TCA], in_=xpt)
                nc.gpsimd.collective_compute(
                    kind="AllReduce", op=mybir.AluOpType.add, replica_groups=groups,
                    ins=[d_xdp[tcb, :, :]], outs=[d_xd[tcb, :, :]])

            nle_id = _act_table_id(nc, "natural_log_exp_and_others")

            def phaseB(tcb):
                """dt_proj + softplus for block tcb; write B/C rows to d_bc."""
                t0 = tcb * TCC
                xdt = work.tile([96, TCC], F32, tag="xdt")
                nc.sync.dma_start(out=xdt, in_=d_xd[tcb, :, :])
                xdb = work.tile([96, TCC], BF16, tag="xdb")
                nc.scalar.copy(xdb, xdt)
                nc.sync.dma_start(out=d_bc[:, t0:t0 + TCC], in_=xdb[DTR:DTR + 2 * DS, :])
                # all 4 Exps first, then all 4 Lns (avoids ACT table thrash)
                spe4 = work.tile([128, NDH, 2, TCA], BF16, tag="spe4")
                for h in range(NDH):
                    for j in range(2):
                        pdt = psA.tile([128, TCA], F32, tag="ps")
                        nc.tensor.matmul(pdt, lhsT=dtw[:, h * 128:(h + 1) * 128],
                                         rhs=xdb[0:DTR, j * TCA:(j + 1) * TCA],
                                         start=True, stop=True)
                        nc.scalar.activation(spe4[:, h, j, :], pdt, AF.Exp,
                                             bias=dtb[:, h, 0:1], scale=1.0)
                for h in range(NDH):
                    for j in range(2):
                        nc.scalar.activation(dts[:, h, t0 + j * TCA:t0 + (j + 1) * TCA],
                                             spe4[:, h, j, :], AF.Ln, bias=1.0, scale=1.0)

            def phaseC_head(tcb):
                """D-skip matmuls + dtx + first n-pairs of block tcb (emitted
                before phaseB so the scan pipeline never starves)."""
                t0 = tcb * TCC
                # pin the Exp+Ln table for this round (C exps + B softplus);
                # the auto-chooser would thrash exp_and_others<->natural_log
                nc.scalar.add_instruction(mybir.InstLoadActFuncSet(
                    name=nc.get_next_instruction_name(),
                    act_func_set_id=nle_id, ins=[], outs=[]))
                dtx2 = [None, None]
                for h in range(NDH):
                    for qq in range(2):
                        nc.tensor.matmul(pys[h][qq], lhsT=ddg[:, h, :],
                                         rhs=xs[:, h, t0 + qq * 512:t0 + (qq + 1) * 512],
                                         start=True, stop=False)
                    dtx2[h] = workC.tile([128, 2, TCC], BF16, tag="dtx2",
                                         name=f"dtx2_{tcb}_{h}")
                    nc.vector.tensor_mul(dtx2[h][:, 0, :], dts[:, h, t0:t0 + TCC],
                                         xs[:, h, t0:t0 + TCC])
                    nc.vector.tensor_copy(dtx2[h][:, 1, :], dtx2[h][:, 0, :])
                phaseC_pairs(tcb, dtx2, 0, CHEAD)
                return dtx2

            def phaseC_pairs(tcb, dtx2, p0, p1):
                t0 = tcb * TCC
                for np_ in range(p0, p1):
                    n0 = 2 * np_
                    bbc2 = bcp.tile([128, 2, TCC], BF16, tag="bbc2")
                    cbc2 = bcp.tile([128, 2, TCC], BF16, tag="cbc2")
                    for s in range(2):
                        nc.sync.dma_start(
                            out=bbc2[:, s, :],
                            in_=bass.AP(tensor=d_bc[:, :].tensor,
                                        offset=(n0 + s) * T + t0,
                                        ap=[[0, 128], [1, TCC]]))
                        nc.sync.dma_start(
                            out=cbc2[:, s, :],
                            in_=bass.AP(tensor=d_bc[:, :].tensor,
                                        offset=(DS + n0 + s) * T + t0,
                                        ap=[[0, 128], [1, TCC]]))
                    for h in range(NDH):
                        dBx2 = workC.tile([128, 2, TCC], BF16, tag="dBx2")
                        if np_ < NPP:
                            nc.gpsimd.tensor_mul(dBx2, dtx2[h], bbc2)
                        else:
                            nc.vector.tensor_mul(dBx2, dtx2[h], bbc2)
                        hts2 = workC.tile([128, 2, TCC], BF16, tag="hts2")
                        for s in range(2):
                            n = n0 + s
                            dA = dAp.tile([128, TCC], BF16, tag="dA")
                            nc.scalar.activation(dA, dts[:, h, t0:t0 + TCC], AF.Exp,
                                                 bias=0.0, scale=aneg[:, h, n:n + 1])
                            init = 0.0 if (t0 % L == 0) else carry[:, h, n:n + 1]
                            nc.vector.tensor_tensor_scan(
                                out=hts2[:, s, :], data0=dA, data1=dBx2[:, s, :],
                                initial=init,
                                op0=mybir.AluOpType.mult, op1=mybir.AluOpType.add)
                        if (t0 + TCC) % L != 0:
                            nc.vector.tensor_copy(carry[:, h, n0:n0 + 2],
                                                  hts2[:, :, TCC - 1:TCC])
                        yp2 = workC.tile([128, 2, TCC], BF16, tag="yp2")
                        if np_ < NPP:
                            nc.gpsimd.tensor_mul(yp2, hts2, cbc2)
                        else:
                            nc.vector.tensor_mul(yp2, hts2, cbc2)
                        for s in range(2):
                            for qq in range(2):
                                nc.tensor.matmul(
                                    pys[h][qq], lhsT=ident,
                                    rhs=yp2[:, s, qq * 512:(qq + 1) * 512],
                                    start=False,
                                    stop=(np_ == DS // 2 - 1 and s == 1))

            def phaseC_tail(tcb, dtx2):
                """remaining n-pairs + gate + A2A input write."""
                t0 = tcb * TCC
                phaseC_pairs(tcb, dtx2, CHEAD, DS // 2)
                for h in range(NDH):
                    ybase = workC.tile([128, TCC], BF16, tag="ybase")
                    for qq in range(2):
                        nc.scalar.copy(ybase[:, qq * 512:(qq + 1) * 512], pys[h][qq])
                    zs = workC.tile([128, TCC], BF16, tag="zs")
                    nc.sync.dma_start(out=zs, in_=d_zsp[h, :, t0:t0 + TCC])
                    yg = workC.tile([128, TCC], BF16, tag="yg")
                    nc.vector.tensor_mul(yg, ybase, zs)
                    nc.sync.dma_start(out=d_a2ai[h][tcb, :, :], in_=yg)

            # ---- software pipeline over t-blocks -------------------------
            # emit order per round: C-head(r-2) (instantly-ready scan work),
            # B(r-1) (its dep chain resolves under the head), C-tail(r-2),
            # then A(r) so A's ACT/TE work queues BEHIND scan-critical ops.
            for r in range(NTCB + 2):
                dtx2 = phaseC_head(r - 2) if r >= 2 else None
                if 1 <= r <= NTCB:
                    phaseB(r - 1)
                if r >= 2:
                    phaseC_tail(r - 2, dtx2)
                if r < NTCB:
                    phaseA(r)

        # ---- AllToAll (split by channel half; E's first half overlaps
        # the second transfer) ---------------------------------------------
        for h in range(NDH):
            nc.gpsimd.collective_compute(
                kind="AllToAll", op=mybir.AluOpType.bypass, replica_groups=groups,
                ins=[d_a2ai[h][:, :, :]], outs=[d_a2ao[h][:, :, :]])

        # ---- out_proj: 2 dm-half passes, 8 PSUM accumulators.
        # ykt tiles and all weights are cached in SBUF (body pools are
        # closed); the weight loads overlap the AllToAll.
        with tc.tile_pool(name="cacheE", bufs=1) as cacheE, \
             tc.tile_pool(name="workE", bufs=2) as workE, \
             tc.tile_pool(name="psE", bufs=1, space="PSUM") as psE:
            pos = [psE.tile([128, 512], F32, tag=f"pe_{ts}", name=f"pe_{ts}")
                   for ts in range(8)]
            wots = cacheE.tile([128, 2 * NCORES, DM], BF16, tag="wots")
            nc.sync.dma_start(
                out=wots, in_=d_woutT[:, :, :].rearrange("k p m -> p k m"))
            ykts = []
            for fh in range(2):
                for kt in range(2 * NCORES):
                    h, i = kt // NCORES, kt % NCORES
                    if fh == 0:
                        ykt = cacheE.tile([128, TSL], BF16, tag=f"ykS{kt}",
                                          name=f"ykS{kt}")
                        nc.sync.dma_start(out=ykt, in_=d_a2ao[h][i, :, :])
                        ykts.append(ykt)
                    else:
                        ykt = ykts[kt]
                    for ts in range(8):
                        nc.tensor.matmul(
                            pos[ts], lhsT=ykt[:, ts * 128:(ts + 1) * 128],
                            rhs=wots[:, 2 * i + h, fh * 512:(fh + 1) * 512],
                            start=(kt == 0), stop=(kt == 15))
                for ts in range(8):
                    ot = workE.tile([128, 512], F32, tag="otS")
                    nc.scalar.copy(ot, pos[ts])
                    nc.sync.dma_start(
                        out=d_out[ts * 128:(ts + 1) * 128, fh * 512:(fh + 1) * 512],
                        in_=ot)

    nc.compile()
    return nc


def _host_prep(inputs):
    """Per-core input maps from full inputs (layout prep + bf16 casts only)."""
    hs = np.asarray(inputs["hidden_states"], np.float32)
    wxz = np.asarray(inputs["in_proj_w"], np.float32)
    cw = np.asarray(inputs["conv_w"], np.float32)
    cb = np.asarray(inputs["conv_b"], np.float32)
    xpw = np.asarray(inputs["x_proj_w"], np.float32)
    dpw = np.asarray(inputs["dt_proj_w"], np.float32)
    dpb = np.asarray(inputs["dt_proj_b"], np.float32)
    alog = np.asarray(inputs["A_log"], np.float32)
    dv = np.asarray(inputs["D"], np.float32)
    wo = np.asarray(inputs["out_proj_w"], np.float32)

    hT = np.ascontiguousarray(hs.reshape(T, DM).T).astype(BF)
    woutT = np.ascontiguousarray(wo.T).reshape(2 * NCORES, 128, DM).astype(BF)
    ident = np.eye(128, dtype=np.float32).astype(BF)

    in_maps = []
    for i in range(NCORES):
        lo = i * DL
        sl = slice(lo, lo + DL)
        wxzT = np.ascontiguousarray(
            np.concatenate([wxz[sl], wxz[DI + lo:DI + lo + DL]], axis=0).T).astype(BF)
        cdiag = np.zeros((DC, NDH, 128, 128), np.float32)
        for j in range(DC):
            for h in range(NDH):
                np.fill_diagonal(cdiag[j, h], cw[lo + h * 128:lo + (h + 1) * 128, j])
        ddiag = np.zeros((NDH, 128, 128), np.float32)
        for h in range(NDH):
            np.fill_diagonal(ddiag[h], dv[lo + h * 128:lo + (h + 1) * 128])
        in_maps.append({
            "hT": hT,
            "wxzT": wxzT,
            "cdiag": cdiag.astype(BF),
            "convb": cb[sl].reshape(NDH, 128, 1),
            "xprojT": np.ascontiguousarray(xpw[:, sl].T).reshape(NDH, 128, 96).astype(BF),
            "dtwT": np.ascontiguousarray(dpw[sl].T).astype(BF),
            "dtb": dpb[sl].reshape(NDH, 128, 1),
            "aneg": (-np.exp(alog[sl])).reshape(NDH, 128, DS).astype(np.float32),
            "ddiag": ddiag.astype(BF),
            "woutT": woutT,
            "ident": ident,
        })
    return in_maps


def _run(inputs, trace=False, **kw):
    if "nc" not in _cached:
        _cached["nc"] = _build()
    nc = _cached["nc"]
    in_maps = _host_prep(inputs)
    res = bass_utils.run_bass_kernel_spmd(
        nc, in_maps, core_ids=list(range(NCORES)), trace=trace, **kw)
    out = np.concatenate([res.results[i]["out_slice"] for i in range(NCORES)], axis=0)
    return out.reshape(B, L, DM).astype(np.float32), res


def kernel(**inputs):
    out, _ = _run(inputs, trace=False)
    return out
